# revision 14
# baseline (speedup 1.0000x reference)
"""Trainium2 Bass kernel for 2-layer GAT (nn_GAT_45157286150549) — v3b.

8-core SPMD, edge partitioning by dst ownership. Key design vs v1 baseline:
- Layer-1 table is bf16 (768B rows vs 1280B) with heads INTERLEAVED
  (column c*8+h) so the per-edge exp*h multiply runs in the DVE 2x mode.
- One-hot selection matrices Pm (aggregation) and PmT (a_dst selection)
  are host-built and streamed in as fp8e4m3 — no per-channel DVE
  is_equal / PE transpose / ACT PSUM copy chains.  (fp8 lhsT x bf16 rhs
  matmul verified exact on HW.)
- e = a_src + a_dst + lrelu + exp chains are quad-batched (4 channels per
  instruction) to amortize ACT/DVE fixed overheads; lrelu uses Prelu
  (parametric_relu), which shares the ACT table set with Exp (no reloads).
- Node ids globally permuted (AllGather-chunk-major) so both layers share
  identical gather indices / one-hot inputs, and the chunked AllGather
  writes contiguous h2A slices while overlapping phase B compute.
"""

import os
import sys

for _p in ("/opt/trn_rl_repo", "/root/.axon_site/_ro/trn_rl_repo"):
    if os.path.isdir(_p) and _p not in sys.path:
        sys.path.insert(0, _p)

import numpy as np
import ml_dtypes

import concourse.bass as bass
import concourse.bacc as bacc
import concourse.mybir as mybir
import concourse.tile as tile
from concourse.library_config import mlp
from concourse.tile import add_dep_helper
from concourse.bass_utils import run_bass_kernel_spmd

# ---------------- problem constants ----------------
N, F_IN, E = 50000, 128, 800000
HID, HEADS, EMB = 32, 8, 64
NEG_SLOPE = 0.2

NCORES = 8
P = 128
NB = 49                    # node blocks per core
NODES_PC = NB * P          # 6272
NTOT = NCORES * NODES_PC   # 50176
LO = 32768                 # int16 gather-index split
TW = 384                   # layer-1 table row width (264 used, 768B)

F32 = mybir.dt.float32
BF16 = mybir.dt.bfloat16
I16 = mybir.dt.int16

CHUNKS = [(0, 20), (20, 34), (34, 44), (44, 49)]
NBG = int(os.environ.get("GAT_NBG", "2"))       # blocks per gather group
GROUPS = [(b0, min(b0 + NBG, NB)) for b0 in range(0, NB, NBG)]

PM_DT = os.environ.get("GAT_PM_DT", "fp8")      # fp8 | bf16
PMDT = mybir.dt.float8e4 if PM_DT == "fp8" else BF16
PM_NP = ml_dtypes.float8_e4m3fn if PM_DT == "fp8" else ml_dtypes.bfloat16
PHASES = os.environ.get("GAT_PHASES", "abgc")

SB_BUFS = int(os.environ.get("GAT_SB_BUFS", "4"))
G_BUFS = int(os.environ.get("GAT_G_BUFS", "2"))
QB = 4                                          # e-chain batch (channels)
MAX_IDX_CH = int(os.environ.get("GAT_MAX_IDX_CH", "8"))
LREL_NAME = os.environ.get("GAT_LREL", "prelu")

# interleave permutation: new col j <- old col (j%8)*32 + j//8
ILV = (np.arange(256) % 8) * 32 + np.arange(256) // 8


def _node_perm():
    """inv[orig node id] = mapped (chunk-major) row id."""
    order = []
    for b0, b1 in CHUNKS:
        for c in range(NCORES):
            base = c * NODES_PC + b0 * P
            order.append(np.arange(base, base + (b1 - b0) * P))
    perm = np.concatenate(order)
    inv = np.empty(NTOT, np.int64)
    inv[perm] = np.arange(NTOT)
    return inv


NODE_MAP = _node_perm()
CHUNK_ROW0 = []
acc = 0
for b0, b1 in CHUNKS:
    CHUNK_ROW0.append(acc)
    acc += NCORES * (b1 - b0) * P
assert acc == NTOT


def _idx_stream(flat_i16: np.ndarray) -> np.ndarray:
    """[L] int16 -> [128, L//16]: element (p, s) = flat[s*16 + p%16]."""
    L = len(flat_i16)
    a16 = flat_i16.reshape(L // 16, 16).T
    return np.tile(a16, (8, 1)).astype(np.int16)


# ============================================================
# Device program
# ============================================================

def build_nc(nlo_b, nhi_b, b1z, b2z):
    nlo_b, nhi_b = list(nlo_b), list(nhi_b)
    nch_b = [a + b for a, b in zip(nlo_b, nhi_b)]
    CT = sum(nch_b)
    CLO = sum(nlo_b)
    CHI = sum(nhi_b)

    grp_info = []   # (ct_off, clo_off, chi_off, nlo_g, nhi_g, col_of)
    ct_off = clo_off = chi_off = 0
    for b0, b1 in GROUPS:
        nlo_g = sum(nlo_b[b0:b1])
        nhi_g = sum(nhi_b[b0:b1])
        col_of = {}
        lo_acc, hi_acc = 0, 0
        for b in range(b0, b1):
            for j in range(nch_b[b]):
                if j < nlo_b[b]:
                    col_of[(b, j)] = lo_acc + j
                else:
                    col_of[(b, j)] = nlo_g + hi_acc + (j - nlo_b[b])
            lo_acc += nlo_b[b]
            hi_acc += nhi_b[b]
        grp_info.append((ct_off, clo_off, chi_off, nlo_g, nhi_g, col_of))
        ct_off += nlo_g + nhi_g
        clo_off += nlo_g
        chi_off += nhi_g

    nc = bacc.Bacc("TRN2", target_bir_lowering=False, debug=False,
                   num_devices=NCORES)
    dt = nc.dram_tensor
    xallT = dt("xallT", [F_IN, NTOT], BF16, kind="ExternalInput").ap()
    xownT = dt("xownT", [F_IN, NODES_PC], BF16, kind="ExternalInput").ap()
    W1c = dt("W1c", [F_IN, 264], BF16, kind="ExternalInput").ap()
    Ad1 = dt("Ad1", [F_IN, 8], BF16, kind="ExternalInput").ap()
    W2c = dt("W2c", [2, P, 66], BF16, kind="ExternalInput").ap()
    b1t = dt("b1t", [P, 256], F32, kind="ExternalInput").ap()
    b2t = dt("b2t", [P, 64], F32, kind="ExternalInput").ap()
    identf = dt("identf", [P, P], F32, kind="ExternalInput").ap()
    pmD = dt("pm", [P, CT * P], PMDT, kind="ExternalInput").ap()
    pmtD = dt("pmt", [P, CT * P], PMDT, kind="ExternalInput").ap()
    ilD = dt("il", [P, CLO * 8], I16, kind="ExternalInput").ap()
    ihD = dt("ih", [P, CHI * 8], I16, kind="ExternalInput").ap()

    out2 = dt("out2", [NODES_PC, EMB], F32, kind="ExternalOutput").ap()

    hA = dt("hA", [NTOT, TW], BF16).ap()
    cc_cs = [dt(f"cc_in{i}", [(b1 - b0) * P, P], BF16).ap()
             for i, (b0, b1) in enumerate(CHUNKS)]
    h2A = dt("h2A", [NTOT, P], BF16, addr_space="Shared").ap()

    with tile.TileContext(nc) as tc:
        with (
            tc.tile_pool(name="const", bufs=1) as cp,
            tc.tile_pool(name="persist", bufs=1) as pp,
        ):
            lib_inst = nc.gpsimd.load_library(mlp)

            def gather(**kw):
                g = nc.gpsimd.dma_gather(**kw)
                add_dep_helper(g.ins, lib_inst.ins, sync=True,
                               reason="mlp library before gather")
                return g

            w1_sb = cp.tile([F_IN, 264], BF16)
            nc.sync.dma_start(out=w1_sb[:], in_=W1c[:])
            ad1_sb = cp.tile([F_IN, 8], BF16)
            nc.sync.dma_start(out=ad1_sb[:], in_=Ad1[:])
            w2a_sb = cp.tile([P, 66], BF16, tag="w2a")
            nc.sync.dma_start(out=w2a_sb[:], in_=W2c[0])
            w2b_sb = cp.tile([P, 66], BF16, tag="w2b")
            nc.sync.dma_start(out=w2b_sb[:], in_=W2c[1])
            identf_sb = cp.tile([P, P], F32, tag="identf")
            nc.sync.dma_start(out=identf_sb[:], in_=identf[:])
            if not b1z:
                b1_sb = cp.tile([P, 256], F32)
                nc.sync.dma_start(out=b1_sb[:], in_=b1t[:])
            if not b2z:
                b2_sb = cp.tile([P, 64], F32)
                nc.sync.dma_start(out=b2_sb[:], in_=b2t[:])

            a2bf = pp.tile([P, NB], BF16)

            LREL = (mybir.ActivationFunctionType.Relu
                    if LREL_NAME == "relu" else
                    mybir.ActivationFunctionType.Prelu)
            EXPF = mybir.ActivationFunctionType.Exp

            # ---------------- Phase A: build layer-1 table ----------------
            with (
                tc.tile_pool(name="pa_sb", bufs=3) as pa,
                tc.tile_pool(name="pa_ps", bufs=2, space="PSUM") as paps,
            ):
                for m0 in range(0, NTOT // P if "a" in PHASES else 0, 2):
                    xs = pa.tile([F_IN, 2 * P], BF16, tag="xs")
                    nc.sync.dma_start(
                        out=xs[:], in_=xallT[:, m0 * P:(m0 + 2) * P])
                    psA = paps.tile([P, 2, 512], F32, tag="psA")
                    for k in range(2):
                        nc.tensor.matmul(
                            psA[:, k, 0:264], lhsT=xs[:, k * P:(k + 1) * P],
                            rhs=w1_sb[:], start=True, stop=True)
                    hbf = pa.tile([P, 2, 264], BF16, tag="hbf")
                    nc.scalar.copy(out=hbf[:], in_=psA[:, :, 0:264])
                    dst_rows = bass.AP(
                        hA.tensor, m0 * P * TW,
                        [[TW, P], [TW * P, 2], [1, 264]])
                    nc.gpsimd.dma_start(out=dst_rows, in_=hbf[:])

            # ---------------- Phase B: layer-1 edge pass ------------------
            with (
                tc.tile_pool(name="pb_g", bufs=G_BUFS) as pg,
                tc.tile_pool(name="pb_sb", bufs=SB_BUFS) as pb,
                tc.tile_pool(name="pb_exm", bufs=SB_BUFS) as px,
                tc.tile_pool(name="pb_pe", bufs=2, space="PSUM") as ppe,
                tc.tile_pool(name="pb_agg", bufs=1, space="PSUM") as pagg,
                tc.tile_pool(name="pb_blk", bufs=1, space="PSUM") as pblk,
            ):
                for gi, (b0, b1) in enumerate(GROUPS if "b" in PHASES else []):
                    ct0, clo0, chi0, nlo_g, nhi_g, col_of = grp_info[gi]
                    nch_g = nlo_g + nhi_g
                    il = pb.tile([P, nlo_g * 8], I16, tag="il")
                    nc.sync.dma_start(
                        out=il[:], in_=ilD[:, clo0 * 8:(clo0 + nlo_g) * 8])
                    ih = pb.tile([P, nhi_g * 8], I16, tag="ih")
                    nc.sync.dma_start(
                        out=ih[:], in_=ihD[:, chi0 * 8:(chi0 + nhi_g) * 8])
                    pm = pg.tile([P, nch_g * P], PMDT, tag="pm")
                    nc.sync.dma_start(
                        out=pm[:], in_=pmD[:, ct0 * P:(ct0 + nch_g) * P])
                    pmt = pg.tile([P, nch_g * P], PMDT, tag="pmt")
                    nc.scalar.dma_start(
                        out=pmt[:], in_=pmtD[:, ct0 * P:(ct0 + nch_g) * P])
                    xo = pb.tile([F_IN, (b1 - b0) * P], BF16, tag="xo")
                    nc.sync.dma_start(
                        out=xo[:], in_=xownT[:, b0 * P:b1 * P])

                    G = pg.tile([P, nch_g * TW], BF16, tag="G")
                    g3 = G[:].rearrange("p (c e) -> p c e", e=TW)
                    for nch_s, idxt, tab, coff in (
                            (nlo_g, il, hA[0:LO, :], 0),
                            (nhi_g, ih, hA[LO:NTOT, :], nlo_g)):
                        for c0 in range(0, nch_s, MAX_IDX_CH):
                            cn = min(MAX_IDX_CH, nch_s - c0)
                            gather(
                                out_ap=g3[:, coff + c0:coff + c0 + cn, :],
                                in_ap=tab,
                                idxs_ap=idxt[:, c0 * 8:(c0 + cn) * 8],
                                num_idxs=cn * P, num_idxs_reg=cn * P,
                                elem_size=TW)

                    for b in range(b0, b1):
                        nch = nch_b[b]
                        pa1 = pblk.tile([P, 8], F32, tag="pa1")
                        nc.tensor.matmul(
                            pa1[:], lhsT=xo[:, (b - b0) * P:(b - b0 + 1) * P],
                            rhs=ad1_sb[:], start=True, stop=True)
                        a1 = pb.tile([P, 8], BF16, tag="a1")
                        nc.scalar.copy(out=a1[:], in_=pa1[:])

                        nlo = nlo_b[b]
                        # quads must not straddle the lo/hi section (their
                        # G columns would not be adjacent)
                        quads = []
                        for s0, s1 in ((0, nlo), (nlo, nch)):
                            for j0 in range(s0, s1, QB):
                                quads.append(
                                    list(range(j0, min(j0 + QB, s1))))
                        agg = pagg.tile([P, 264], F32, tag="agg")
                        pend = None
                        for js in quads:
                            nk = len(js)
                            pe1 = ppe.tile([P, QB, 8], F32, tag="pe1")
                            exm = px.tile([P, QB, 264], BF16, tag="exm")
                            ee = pb.tile([P, QB, 8], F32, tag="ee")
                            e2 = pb.tile([P, QB, 8], F32, tag="e2")
                            for k, j in enumerate(js):
                                col = col_of[(b, j)] * P
                                nc.tensor.matmul(
                                    pe1[:, k, :],
                                    lhsT=pmt[:, col:col + P],
                                    rhs=a1[:], start=(k == 0),
                                    stop=(k == nk - 1),
                                    skip_group_check=True)
                            cols = [col_of[(b, j)] for j in js]
                            assert cols == list(
                                range(cols[0], cols[0] + nk)) or nk == 1, cols
                            # ga (gathered a_src) + a_dst
                            nc.vector.tensor_tensor(
                                out=ee[:, 0:nk, :],
                                in0=g3[:, cols[0]:cols[0] + nk, 256:264],
                                in1=pe1[:, 0:nk, :],
                                op=mybir.AluOpType.add)
                            nc.scalar.activation(
                                out=e2[:, 0:nk, :], in_=ee[:, 0:nk, :],
                                func=LREL, alpha=NEG_SLOPE)
                            nc.scalar.activation(
                                out=exm[:, 0:nk, 0:8], in_=e2[:, 0:nk, :],
                                func=EXPF)
                            nc.vector.tensor_tensor(
                                out=exm[:, 0:nk, 8:264].rearrange(
                                    "p b (c h) -> p b c h", h=8),
                                in0=g3[:, cols[0]:cols[0] + nk, 0:256]
                                .rearrange("p b (c h) -> p b c h", h=8),
                                in1=exm[:, 0:nk, 0:8].rearrange(
                                    "p b (c h) -> p b c h", c=1
                                ).to_broadcast([P, nk, 32, 8]),
                                op=mybir.AluOpType.mult)
                            if pend is not None:
                                for k, j in pend:
                                    col = col_of[(b, j)] * P
                                    nc.tensor.matmul(
                                        agg[:], lhsT=pm[:, col:col + P],
                                        rhs=pend_exm[:, k, :],
                                        start=(j == 0), stop=(j == nch - 1))
                            pend = list(zip(range(nk), js))
                            pend_exm = exm
                        for k, j in pend:
                            col = col_of[(b, j)] * P
                            nc.tensor.matmul(
                                agg[:], lhsT=pm[:, col:col + P],
                                rhs=pend_exm[:, k, :],
                                start=(j == 0), stop=(j == nch - 1))

                        # ---- block finals (h1 is head-interleaved) ----
                        den = pb.tile([P, 8], F32, tag="den")
                        nc.vector.tensor_scalar_add(den[:], agg[:, 0:8], 1e-16)
                        R = pb.tile([P, 8], F32, tag="R")
                        nc.vector.reciprocal(R[:], den[:])
                        h1 = pb.tile([P, 256], F32, tag="h1")
                        nc.vector.tensor_tensor(
                            out=h1[:].rearrange("p (c h) -> p c h", h=8),
                            in0=agg[:, 8:264].rearrange(
                                "p (c h) -> p c h", h=8),
                            in1=R[:].rearrange("p (c h) -> p c h", c=1)
                            .to_broadcast([P, 32, 8]),
                            op=mybir.AluOpType.mult)
                        if not b1z:
                            nc.vector.tensor_add(out=h1[:], in0=h1[:],
                                                 in1=b1_sb[:])
                        nc.vector.tensor_scalar_max(h1[:], h1[:], 0.0)

                        # ---- layer-2 node compute ----
                        ps2 = pblk.tile([P, 66], F32, tag="ps2")
                        for k in range(2):
                            tp = pblk.tile([P, P], F32, tag="tp")
                            nc.tensor.transpose(
                                out=tp[:], in_=h1[:, k * P:(k + 1) * P],
                                identity=identf_sb[:])
                            hT = pb.tile([P, P], BF16, tag="hT")
                            nc.scalar.copy(out=hT[:], in_=tp[:])
                            nc.tensor.matmul(
                                ps2[:], lhsT=hT[:],
                                rhs=(w2a_sb[:] if k == 0 else w2b_sb[:]),
                                start=(k == 0), stop=(k == 1))
                        cbf = pb.tile([P, 67], BF16, tag="cbf")
                        nc.vector.memset(cbf[:, 0:1], 1.0)
                        nc.scalar.copy(out=cbf[:, 1:67], in_=ps2[:, 0:66])
                        nc.scalar.copy(out=a2bf[:, b:b + 1],
                                       in_=ps2[:, 65:66])
                        ci = next(i for i, (c0_, c1_) in enumerate(CHUNKS)
                                  if c0_ <= b < c1_)
                        cb0 = CHUNKS[ci][0]
                        nc.sync.dma_start(
                            out=cc_cs[ci][(b - cb0) * P:(b - cb0 + 1) * P,
                                          0:67],
                            in_=cbf[:])

                    if "g" in PHASES:
                        for ci, (cb0, cb1) in enumerate(CHUNKS):
                            if b1 == cb1:
                                r0 = CHUNK_ROW0[ci]
                                nrow = NCORES * (cb1 - cb0) * P
                                nc.gpsimd.collective_compute(
                                    "AllGather", mybir.AluOpType.bypass,
                                    replica_groups=[list(range(NCORES))],
                                    ins=[cc_cs[ci][:].opt()],
                                    outs=[h2A[r0:r0 + nrow, :].opt()])

            # ---------------- Phase C: layer-2 edge pass ----------------
            with (
                tc.tile_pool(name="pc_g", bufs=G_BUFS) as pg2,
                tc.tile_pool(name="pc_sb", bufs=SB_BUFS) as pc,
                tc.tile_pool(name="pc_exm", bufs=SB_BUFS) as px2,
                tc.tile_pool(name="pc_pe", bufs=2, space="PSUM") as ppe2,
                tc.tile_pool(name="pc_agg", bufs=2, space="PSUM") as pagg2,
            ):
                for gi, (b0, b1) in enumerate(GROUPS if "c" in PHASES else []):
                    ct0, clo0, chi0, nlo_g, nhi_g, col_of = grp_info[gi]
                    nch_g = nlo_g + nhi_g
                    il = pc.tile([P, nlo_g * 8], I16, tag="il")
                    nc.sync.dma_start(
                        out=il[:], in_=ilD[:, clo0 * 8:(clo0 + nlo_g) * 8])
                    ih = pc.tile([P, nhi_g * 8], I16, tag="ih")
                    nc.sync.dma_start(
                        out=ih[:], in_=ihD[:, chi0 * 8:(chi0 + nhi_g) * 8])
                    pm = pg2.tile([P, nch_g * P], PMDT, tag="pm")
                    nc.sync.dma_start(
                        out=pm[:], in_=pmD[:, ct0 * P:(ct0 + nch_g) * P])
                    pmt = pg2.tile([P, nch_g * P], PMDT, tag="pmt")
                    nc.scalar.dma_start(
                        out=pmt[:], in_=pmtD[:, ct0 * P:(ct0 + nch_g) * P])

                    G2 = pg2.tile([P, nch_g * P], BF16, tag="G2")
                    g3 = G2[:].rearrange("p (c e) -> p c e", e=P)
                    for nch_s, idxt, tab, coff in (
                            (nlo_g, il, h2A[0:LO, :], 0),
                            (nhi_g, ih, h2A[LO:NTOT, :], nlo_g)):
                        for c0 in range(0, nch_s, MAX_IDX_CH):
                            cn = min(MAX_IDX_CH, nch_s - c0)
                            gather(
                                out_ap=g3[:, coff + c0:coff + c0 + cn, :],
                                in_ap=tab,
                                idxs_ap=idxt[:, c0 * 8:(c0 + cn) * 8],
                                num_idxs=cn * P, num_idxs_reg=cn * P,
                                elem_size=P)

                    for b in range(b0, b1):
                        nch = nch_b[b]
                        nlo = nlo_b[b]
                        # quads must not straddle the lo/hi section
                        quads = []
                        for s0, s1 in ((0, nlo), (nlo, nch)):
                            for j0 in range(s0, s1, QB):
                                quads.append(
                                    list(range(j0, min(j0 + QB, s1))))
                        agg2 = pagg2.tile([P, 65], F32, tag="agg2")
                        pend = None
                        for js in quads:
                            nk = len(js)
                            pe2 = ppe2.tile([P, QB, 8], F32, tag="pe2")
                            exm2 = px2.tile([P, QB, 65], BF16, tag="exm2")
                            lr2 = pc.tile([P, QB, 1], F32, tag="lr2")
                            lr2b = pc.tile([P, QB, 1], F32, tag="lr2b")
                            exf2 = pc.tile([P, QB, 1], F32, tag="exf2")
                            for k, j in enumerate(js):
                                col = col_of[(b, j)] * P
                                nc.tensor.matmul(
                                    pe2[:, k, 0:1],
                                    lhsT=pmt[:, col:col + P],
                                    rhs=a2bf[:, b:b + 1],
                                    start=(k == 0), stop=(k == nk - 1),
                                    skip_group_check=True)
                            ji = col_of[(b, js[0])]
                            nc.vector.tensor_tensor(
                                out=lr2[:, 0:nk, :],
                                in0=g3[:, ji:ji + nk, 65:66],
                                in1=pe2[:, 0:nk, 0:1],
                                op=mybir.AluOpType.add)
                            nc.scalar.activation(
                                out=lr2b[:, 0:nk, :], in_=lr2[:, 0:nk, :],
                                func=LREL, alpha=NEG_SLOPE)
                            nc.scalar.activation(
                                out=exf2[:, 0:nk, :], in_=lr2b[:, 0:nk, :],
                                func=EXPF)
                            for k, j in enumerate(js):
                                jc = col_of[(b, j)]
                                nc.vector.tensor_scalar(
                                    out=exm2[:, k, 0:65],
                                    in0=g3[:, jc, 0:65],
                                    scalar1=exf2[:, k, 0:1], scalar2=None,
                                    op0=mybir.AluOpType.mult)
                            if pend is not None:
                                for k, j in pend:
                                    col = col_of[(b, j)] * P
                                    nc.tensor.matmul(
                                        agg2[:], lhsT=pm[:, col:col + P],
                                        rhs=pend_exm[:, k, :],
                                        start=(j == 0), stop=(j == nch - 1))
                            pend = list(zip(range(nk), js))
                            pend_exm = exm2
                        for k, j in pend:
                            col = col_of[(b, j)] * P
                            nc.tensor.matmul(
                                agg2[:], lhsT=pm[:, col:col + P],
                                rhs=pend_exm[:, k, :],
                                start=(j == 0), stop=(j == nch - 1))

                        den2 = pc.tile([P, 1], F32, tag="den2")
                        nc.vector.tensor_scalar_add(
                            den2[:], agg2[:, 0:1], 1e-16)
                        R2 = pc.tile([P, 1], F32, tag="R2")
                        nc.vector.reciprocal(R2[:], den2[:])
                        o2 = pc.tile([P, 64], F32, tag="o2")
                        nc.vector.tensor_scalar(
                            out=o2[:], in0=agg2[:, 1:65],
                            scalar1=R2[:], scalar2=None,
                            op0=mybir.AluOpType.mult)
                        if not b2z:
                            nc.vector.tensor_add(out=o2[:], in0=o2[:],
                                                 in1=b2_sb[:])
                        nc.sync.dma_start(
                            out=out2[b * P:(b + 1) * P, :], in_=o2[:])

    nc.compile()
    return nc


# ============================================================
# Host preprocessing
# ============================================================

def prepare(x, edge_index, W_src1, W_dst1, att_src1, att_dst1, b1,
            W_src2, W_dst2, att_src2, att_dst2, b2):
    x = np.asarray(x, np.float32)
    src = np.asarray(edge_index[0], np.int64)
    dst = np.asarray(edge_index[1], np.int64)

    att1s = np.asarray(att_src1, np.float32)
    att1d = np.asarray(att_dst1, np.float32)
    bd1s = np.zeros((HEADS * HID, HEADS), np.float32)
    bd1d = np.zeros((HEADS * HID, HEADS), np.float32)
    for h in range(HEADS):
        bd1s[h * HID:(h + 1) * HID, h] = att1s[h]
        bd1d[h * HID:(h + 1) * HID, h] = att1d[h]
    A_src1 = np.asarray(W_src1, np.float32) @ bd1s
    A_dst1 = np.asarray(W_dst1, np.float32) @ bd1d
    W1i = np.asarray(W_src1, np.float32)[:, ILV]          # interleaved cols
    W1c = np.concatenate([W1i, A_src1], axis=1)           # [128, 264]

    A_src2 = np.asarray(W_src2, np.float32) @ np.asarray(
        att_src2, np.float32).reshape(EMB, 1)
    A_dst2 = np.asarray(W_dst2, np.float32) @ np.asarray(
        att_dst2, np.float32).reshape(EMB, 1)
    W2c = np.concatenate(
        [np.asarray(W_src2, np.float32), A_src2, A_dst2], axis=1)
    W2c = W2c[ILV]                                        # interleaved rows
    W2c = W2c.reshape(2, P, 66)

    bf = ml_dtypes.bfloat16
    xpad = np.zeros((NTOT, F_IN), np.float32)
    xpad[:N] = x
    xallT = np.zeros((F_IN, NTOT), np.float32)
    xallT[:, NODE_MAP[:N]] = x.T                          # mapped col order

    b1 = np.asarray(b1, np.float32)
    b2 = np.asarray(b2, np.float32)
    b1z = not np.any(b1)
    b2z = not np.any(b2)

    ms_all = NODE_MAP[src]
    owner = dst // NODES_PC

    per_core = []
    for c in range(NCORES):
        m = owner == c
        s_c, d_c = ms_all[m], dst[m] - c * NODES_PC
        blk = d_c // P
        dl = d_c % P
        blocks = []
        for b in range(NB):
            mb = blk == b
            sb_, db_ = s_c[mb], dl[mb]
            lo = sb_ < LO
            blocks.append((sb_[lo], db_[lo], sb_[~lo] - LO, db_[~lo]))
        per_core.append(blocks)

    nlo_b = [max(max((len(per_core[c][b][0]) + P - 1) // P
                     for c in range(NCORES)), 1) for b in range(NB)]
    nhi_b = [max(max((len(per_core[c][b][2]) + P - 1) // P
                     for c in range(NCORES)), 1) for b in range(NB)]
    nch_b = [a + b for a, b in zip(nlo_b, nhi_b)]
    CT, CLO, CHI = sum(nch_b), sum(nlo_b), sum(nhi_b)

    common = {
        "xallT": xallT.astype(bf),
        "W1c": W1c.astype(bf), "Ad1": A_dst1.astype(bf),
        "W2c": W2c.astype(bf),
        "b1t": np.tile(b1[ILV][None, :], (P, 1)),
        "b2t": np.tile(b2[None, :], (P, 1)),
        "identf": np.eye(P, dtype=np.float32),
    }

    in_maps = []
    for c in range(NCORES):
        il_c = np.zeros((P, CLO * 8), np.int16)
        ih_c = np.zeros((P, CHI * 8), np.int16)
        dlm = np.full((CT, P), -1, np.int16)
        ct0 = clo0 = chi0 = 0
        for b0, b1_ in GROUPS:
            nlo_g = sum(nlo_b[b0:b1_])
            nhi_g = sum(nhi_b[b0:b1_])
            lo_acc, hi_acc = 0, 0
            lo_flat = np.zeros(nlo_g * P, np.int16)
            hi_flat = np.zeros(nhi_g * P, np.int16)
            for b in range(b0, b1_):
                s_lo, d_lo, s_hi, d_hi = per_core[c][b]
                o = lo_acc * P
                lo_flat[o:o + len(s_lo)] = s_lo.astype(np.int16)
                rows = ct0 + lo_acc + np.arange(len(d_lo)) // P
                dlm[rows, np.arange(len(d_lo)) % P] = d_lo
                o = hi_acc * P
                hi_flat[o:o + len(s_hi)] = s_hi.astype(np.int16)
                rows = ct0 + nlo_g + hi_acc + np.arange(len(d_hi)) // P
                dlm[rows, np.arange(len(d_hi)) % P] = d_hi
                lo_acc += nlo_b[b]
                hi_acc += nhi_b[b]
            il_c[:, clo0 * 8:(clo0 + nlo_g) * 8] = _idx_stream(lo_flat)
            ih_c[:, chi0 * 8:(chi0 + nhi_g) * 8] = _idx_stream(hi_flat)
            ct0 += nlo_g + nhi_g
            clo0 += nlo_g
            chi0 += nhi_g

        onehot = dlm[:, :, None] == np.arange(P, dtype=np.int16)[None, None]
        pm_c = np.ascontiguousarray(
            onehot.transpose(1, 0, 2)).astype(PM_NP).reshape(P, CT * P)
        pmt_c = np.ascontiguousarray(
            onehot.transpose(2, 0, 1)).astype(PM_NP).reshape(P, CT * P)

        xoT = np.ascontiguousarray(
            xpad[c * NODES_PC:(c + 1) * NODES_PC].T).astype(bf)
        in_maps.append({
            **common,
            "xownT": xoT,
            "pm": pm_c, "pmt": pmt_c,
            "il": il_c, "ih": ih_c,
        })
    return in_maps, tuple(nlo_b), tuple(nhi_b), b1z, b2z


_NC_CACHE = {}


def kernel(**inputs) -> np.ndarray:
    in_maps, nlo_b, nhi_b, b1z, b2z = prepare(**inputs)
    key = (nlo_b, nhi_b, b1z, b2z, PM_DT, PHASES, NBG, MAX_IDX_CH, LREL_NAME)
    if key not in _NC_CACHE:
        _NC_CACHE[key] = build_nc(nlo_b, nhi_b, b1z, b2z)
    nc = _NC_CACHE[key]
    res = run_bass_kernel_spmd(
        nc, in_maps, core_ids=list(range(NCORES)),
        trace=bool(int(os.environ.get("GAT_TRACE", "0"))))
    kernel.last_results = res
    out = np.concatenate(
        [res.results[c]["out2"] for c in range(NCORES)], axis=0)
    return out[:N].astype(np.float32)
